# revision 1
# baseline (speedup 1.0000x reference)
"""Trainium2 Bass kernel for LocalXLAttention (chunk-summed variant).

Math: the reference einsum sums over the chunk index z, so every query
attends to the same three [w, dh] K/V matrices built from chunk sums:
  K_prev = S_k - k_chunk[C-1], K_cur = S_k, K_next = S_k - k_chunk[0]
(and identically for V), where S_k = sum_c k_chunk[c].  The computation
collapses to, per sequence position l and head h:
  attn[l,h,:]  = qp[l,h,:] @ KbigT          (KbigT: [dh, 3w])
  probs        = softmax(attn, axis=-1)
  ctx[l,h,:]   = probs[l,h,:] @ Vbig        (Vbig:  [3w, dh])
  out          = ctx.reshape(L, dm) @ Wc

Sharding: L=4096 is split 512 rows per core across 8 NeuronCores
(data-parallel over the sequence; no collectives).  Each core redundantly
computes the tiny chunk-summed K/V from the full kv input.

The attention pipeline runs fully transposed ([j, l] / [he, l] layouts) so
no on-device transposes of activations are needed; probs normalization is
deferred to the context (an extra all-ones column of Vbig accumulates the
softmax denominator for free).

Matmuls run in float32r (TF32-class PE mode, 1 cycle/row vs 4 for fp32).
"""

import sys
for _p in ('/opt/pypackages', '/opt/trn_rl_repo'):
    if _p not in sys.path:
        sys.path.insert(0, _p)

import numpy as np

import concourse.bass as bass
import concourse.bacc as bacc
import concourse.tile as tile
from concourse import mybir
from concourse.bass_utils import run_bass_kernel_spmd
from concourse.masks import make_identity

F32 = mybir.dt.float32
F32R = mybir.dt.float32r
AF = mybir.ActivationFunctionType

N_CORES = 8
L = 4096          # full sequence
LS = L // N_CORES # 512 rows per core
DM = 1024
NH = 16
DH = 64
W = 512           # chunk width
C = L // W        # 8 chunks
J3 = 3 * W        # 1536 softmax width
NJ = J3 // 128    # 12 j-chunks
DMT = DM // 128   # 8 dm-chunks


def build_nc():
    nc = bacc.Bacc(None, target_bir_lowering=False)

    qT = nc.dram_tensor("qT", [DM, LS], F32R, kind="ExternalInput")
    kvT = nc.dram_tensor("kvT", [DM, L], F32R, kind="ExternalInput")
    Wq = nc.dram_tensor("Wq", [DM, DM], F32R, kind="ExternalInput")
    Wkv = nc.dram_tensor("Wkv", [DM, 2 * DH], F32R, kind="ExternalInput")
    Wc = nc.dram_tensor("Wc", [DM, DM], F32R, kind="ExternalInput")
    out = nc.dram_tensor("out", [LS, DM], F32, kind="ExternalOutput")

    with tile.TileContext(nc) as tc:
        with tc.tile_pool(name="weights", bufs=8) as wpool, \
             tc.tile_pool(name="small", bufs=1) as spool, \
             tc.tile_pool(name="qp", bufs=8) as qpool, \
             tc.tile_pool(name="qpt", bufs=4) as qptpool, \
             tc.tile_pool(name="stream", bufs=2) as stpool, \
             tc.tile_pool(name="kvsum", bufs=8) as kvspool, \
             tc.tile_pool(name="var", bufs=4) as varpool, \
             tc.tile_pool(name="probs", bufs=4) as ppool, \
             tc.tile_pool(name="misc", bufs=2) as mpool, \
             tc.tile_pool(name="dram", bufs=1, space="DRAM") as dpool, \
             tc.tile_pool(name="psacc", bufs=4, space="PSUM") as psacc, \
             tc.tile_pool(name="psmm", bufs=2, space="PSUM") as psmm:

            # ---------- load weights / q ----------
            wq_sb = []
            for d in range(DMT):
                t = wpool.tile([128, DM], F32R, tag="wq", name=f"wq{d}")
                nc.gpsimd.dma_start(out=t, in_=Wq[128 * d:128 * (d + 1), :])
                wq_sb.append(t)
            wkv_sb = []
            for d in range(DMT):
                t = wpool.tile([128, 2 * DH], F32R, tag="wkv", name=f"wkv{d}")
                nc.sync.dma_start(out=t, in_=Wkv[128 * d:128 * (d + 1), :])
                wkv_sb.append(t)
            qt_sb = []
            for d in range(DMT):
                t = qpool.tile([128, LS], F32R, tag="qt", name=f"qt{d}")
                nc.gpsimd.dma_start(out=t, in_=qT[128 * d:128 * (d + 1), :])
                qt_sb.append(t)

            ident = spool.tile([128, 128], F32, tag="ident")
            make_identity(nc, ident)

            # ---------- kv stream: chunk-sum (tree, in place) ----------
            # kvsum_sb[d][p, y] = sum_c kvT[128d+p, 512c + y]
            kvsum_sb = []
            k7p = psacc.tile([128, W], F32, tag="acc", name="k7p")
            v7p = psacc.tile([128, W], F32, tag="acc", name="v7p")
            for d in range(DMT):
                st = stpool.tile([128, L], F32R, tag="kvstream")
                nc.sync.dma_start(out=st[:, 0:L // 2],
                                  in_=kvT[128 * d:128 * (d + 1), 0:L // 2])
                nc.scalar.dma_start(out=st[:, L // 2:L],
                                    in_=kvT[128 * d:128 * (d + 1), L // 2:L])
                # chunk-7 columns [3584:4096] are only read (never written) by
                # the in-place tree below, so project k7/v7 straight from the
                # stream tile instead of reloading those columns later.
                nc.tensor.matmul(k7p[0:DH, :], wkv_sb[d][:, 0:DH],
                                 st[:, L - W:L], start=(d == 0),
                                 stop=(d == DMT - 1))
                nc.tensor.matmul(v7p[0:DH, :], wkv_sb[d][:, DH:2 * DH],
                                 st[:, L - W:L], start=(d == 0),
                                 stop=(d == DMT - 1))
                nc.vector.tensor_add(st[:, 0:2048], st[:, 0:2048], st[:, 2048:4096])
                nc.vector.tensor_add(st[:, 0:1024], st[:, 0:1024], st[:, 1024:2048])
                ks = kvspool.tile([128, W], F32R, tag="kvsum")
                nc.vector.tensor_add(ks, st[:, 0:512], st[:, 512:1024])
                kvsum_sb.append(ks)
            k7_sb = spool.tile([DH, W], F32, tag="k7")
            v7_sb = spool.tile([DH, W], F32, tag="v7")
            nc.vector.tensor_copy(k7_sb, k7p[0:DH, :])
            nc.vector.tensor_copy(v7_sb, v7p[0:DH, :])

            # ---------- QP_T = Wq.T @ q.T  (unscaled; 1/sqrt(dh) folded into exp) ----
            # qpt_sb[t][p, 512*half + l] = QP_T[hd = 128*(2t+half) + p, l]
            qpt_sb = []
            for t4 in range(4):
                ps = psmm.tile([128, 1024], F32, tag="mm")
                for half in range(2):
                    hd = 2 * t4 + half
                    for d in range(DMT):
                        nc.tensor.matmul(
                            ps[:, 512 * half:512 * (half + 1)],
                            wq_sb[d][:, 128 * hd:128 * (hd + 1)],
                            qt_sb[d],
                            start=(d == 0), stop=(d == DMT - 1))
                sb = qptpool.tile([128, 1024], F32R, tag="qpt")
                nc.vector.tensor_copy(sb, ps)
                qpt_sb.append(sb)

            # ---------- chunk-0 / chunk-7 K,V projections ----------
            # reload kvT columns for chunks 0 and 7 (the stream tiles are
            # mutated in place by the tree sum and rotate away).
            def project_variant(rhs_tiles, tag):
                """returns psum tiles (k [64,512], v [64,512]) accumulated
                over the 8 dm chunks of rhs_tiles (each [128, 512])."""
                kp = psacc.tile([128, W], F32, tag="acc")
                vp = psacc.tile([128, W], F32, tag="acc")
                for d in range(DMT):
                    nc.tensor.matmul(kp[0:DH, :], wkv_sb[d][:, 0:DH],
                                     rhs_tiles[d], start=(d == 0),
                                     stop=(d == DMT - 1))
                    nc.tensor.matmul(vp[0:DH, :], wkv_sb[d][:, DH:2 * DH],
                                     rhs_tiles[d], start=(d == 0),
                                     stop=(d == DMT - 1))
                return kp, vp

            kv0_sb = []
            for d in range(DMT):
                t0 = varpool.tile([128, W], F32R, tag="kv07", name=f"kv0_{d}")
                nc.scalar.dma_start(out=t0, in_=kvT[128 * d:128 * (d + 1), 0:W])
                kv0_sb.append(t0)

            k0_ps, v0_ps = project_variant(kv0_sb, "c0")
            # evacuate immediately so the psum slots can rotate
            k0_sb = spool.tile([DH, W], F32, tag="k0")
            v0_sb = spool.tile([DH, W], F32, tag="v0")
            nc.vector.tensor_copy(k0_sb, k0_ps[0:DH, :])
            nc.vector.tensor_copy(v0_sb, v0_ps[0:DH, :])

            ksum_ps, vsum_ps = project_variant(kvsum_sb, "sum")
            vsum_sb = spool.tile([DH, W], F32, tag="vsum")
            nc.vector.tensor_copy(vsum_sb, vsum_ps[0:DH, :])

            # ---------- KbigT [64, 1536] = [prev | cur | next] ----------
            # duplicated into partitions 64:128 so heads whose QP_T rows sit
            # at base partition 64 get a base-matched lhsT.
            kbig = spool.tile([128, J3], F32R, tag="kbig")
            nc.vector.tensor_sub(kbig[0:DH, 0:W], ksum_ps[0:DH, :], k7_sb)
            nc.vector.tensor_copy(kbig[0:DH, W:2 * W], ksum_ps[0:DH, :])
            nc.vector.tensor_sub(kbig[0:DH, 2 * W:3 * W], ksum_ps[0:DH, :], k0_sb)
            nc.vector.tensor_copy(kbig[DH:2 * DH, :], kbig[0:DH, :])

            # ---------- Vbig [128, 12, 65(+pad)] ----------
            # chunk j rows p: j-index 128j + p of the 1536; col 64 = ones
            # (softmax denominator accumulator).
            vbig = spool.tile([128, NJ, 68], F32R, tag="vbig")
            ones_sb = spool.tile([128, 1], F32, tag="ones")
            nc.vector.memset(ones_sb, 1.0)
            for j in range(NJ):
                nc.vector.tensor_copy(vbig[:, j, DH:DH + 1], ones_sb)
            for yt in range(4):
                tps = psacc.tile([128, W], F32, tag="acc")
                tp0 = psacc.tile([128, W], F32, tag="acc")
                tp7 = psacc.tile([128, W], F32, tag="acc")
                sl = slice(128 * yt, 128 * (yt + 1))
                nc.tensor.transpose(tps[:, 0:DH], vsum_sb[:, sl], ident[0:DH, 0:DH])
                nc.tensor.transpose(tp0[:, 0:DH], v0_sb[:, sl], ident[0:DH, 0:DH])
                nc.tensor.transpose(tp7[:, 0:DH], v7_sb[:, sl], ident[0:DH, 0:DH])
                # DVE may read only one PSUM operand: evacuate cur first,
                # then subtract the other transposes against the SBUF copy.
                nc.vector.tensor_copy(vbig[:, 4 + yt, 0:DH], tps[:, 0:DH])
                nc.vector.tensor_sub(vbig[:, 0 + yt, 0:DH], vbig[:, 4 + yt, 0:DH], tp7[:, 0:DH])
                nc.vector.tensor_sub(vbig[:, 8 + yt, 0:DH], vbig[:, 4 + yt, 0:DH], tp0[:, 0:DH])

            # ---------- attention (transposed): QK -> exp -> PV ----------
            # denominator rows go through a DRAM scratch because engine APs
            # need 32-aligned base partitions (can't write row h directly).
            dscratch = dpool.tile([NH, W], F32, name="dscratch")
            ctxu_sb = []  # 8 pair tiles [128, 512]: rows 0:64 head 2t, 64:128 head 2t+1
            for t in range(8):
                ctxu_sb.append(qpool.tile([128, W], F32R, tag="qt", name=f"ctxu{t}"))

            for t in range(8):  # head pairs (2t, 2t+1)
                qpt = qpt_sb[t // 2]
                csl = slice(512 * (t % 2), 512 * (t % 2) + W)
                rhsA = qpt[0:DH, csl]
                rhsB = qpt[DH:2 * DH, csl]
                ctxA = psacc.tile([128, W], F32, tag="acc", name=f"ctxA{t}")
                ctxB = psacc.tile([128, W], F32, tag="acc", name=f"ctxB{t}")
                for j in range(NJ):
                    qk = psmm.tile([128, 1024], F32, tag="mm", name=f"qk{t}_{j}")
                    # row-packed pair: even head on PE rows 0:64, odd head on
                    # rows 64:128 (tile_position auto-derived from base
                    # partitions) -> both matmuls run concurrently.
                    nc.tensor.matmul(qk[:, 0:W],
                                     kbig[0:DH, 128 * j:128 * (j + 1)],
                                     rhsA, start=True, stop=True)
                    nc.tensor.matmul(qk[:, W:2 * W],
                                     kbig[DH:2 * DH, 128 * j:128 * (j + 1)],
                                     rhsB, start=True, stop=True)
                    pr = ppool.tile([128, 1024], F32R, tag="probs", name=f"pr{t}_{j}")
                    nc.scalar.activation(pr, qk, AF.Exp, scale=0.125)
                    nc.tensor.matmul(ctxA[0:DH + 1, :], vbig[:, j, 0:DH + 1],
                                     pr[:, 0:W],
                                     start=(j == 0), stop=(j == NJ - 1))
                    nc.tensor.matmul(ctxB[0:DH + 1, :], vbig[:, j, 0:DH + 1],
                                     pr[:, W:2 * W],
                                     start=(j == 0), stop=(j == NJ - 1))
                for h, ctx_ps in ((2 * t, ctxA), (2 * t + 1, ctxB)):
                    dtmp = mpool.tile([1, W], F32, tag="dtmp", name=f"dtmp{h}", bufs=1)
                    nc.vector.tensor_copy(dtmp, ctx_ps[DH:DH + 1, :])
                    nc.sync.dma_start(out=dscratch[h:h + 1, :], in_=dtmp)
                    nc.vector.tensor_copy(
                        ctxu_sb[h // 2][64 * (h % 2):64 * (h % 2) + DH, :],
                        ctx_ps[0:DH, :])
                if t % 2 == 1:
                    # normalize the 2 pairs (4 heads) whose denominators are
                    # complete; earlier batches overlap later pairs' compute.
                    b0 = 4 * (t // 2)
                    dn = mpool.tile([4, W], F32, tag="dn", name=f"dn{t}", bufs=1)
                    nc.scalar.dma_start(out=dn, in_=dscratch[b0:b0 + 4, :])
                    rc = mpool.tile([4, W], F32, tag="rc", name=f"rc{t}", bufs=1)
                    nc.vector.reciprocal(rc, dn)
                    rsc = dpool.tile([4, W], F32, name=f"rsc{t}")
                    nc.scalar.dma_start(out=rsc, in_=rc)
                    for pt in (t - 1, t):
                        bc = mpool.tile([128, W], F32, tag="bcast", name=f"bc{pt}")
                        src = bass.AP(tensor=rsc.tensor,
                                      offset=rsc.offset + (2 * pt - b0) * W,
                                      ap=[[W, 2], [0, DH], [1, W]])
                        nc.scalar.dma_start(out=bc, in_=src)
                        nc.vector.tensor_mul(ctxu_sb[pt], ctxu_sb[pt], bc)

            # ---------- out = ctx @ Wc ----------
            wc_sb = []
            for d in range(DMT):
                t = wpool.tile([128, DM], F32R, tag="wc", name=f"wc{d}")
                nc.gpsimd.dma_start(out=t, in_=Wc[128 * d:128 * (d + 1), :])
                wc_sb.append(t)

            for lt in range(LS // 128):
                ps = psmm.tile([128, 1024], F32, tag="mm")
                for half in range(2):
                    for he in range(DMT):
                        nc.tensor.matmul(
                            ps[:, 512 * half:512 * (half + 1)],
                            ctxu_sb[he][:, 128 * lt:128 * (lt + 1)],
                            wc_sb[he][:, 512 * half:512 * (half + 1)],
                            start=(he == 0), stop=(he == DMT - 1))
                ob = mpool.tile([128, DM], F32, tag="outsb", bufs=1)
                nc.vector.tensor_copy(ob, ps)
                nc.sync.dma_start(out=out[128 * lt:128 * (lt + 1), :], in_=ob)

    nc.compile()
    return nc


_NC = None


def _get_nc():
    global _NC
    if _NC is None:
        _NC = build_nc()
    return _NC


def kernel(q, kv, Wq, Wkv, Wc, w):
    assert int(w) == W
    q = np.asarray(q, dtype=np.float32)
    kv = np.asarray(kv, dtype=np.float32)
    B = q.shape[0]
    assert B == 1 and q.shape[1] == L and q.shape[2] == DM

    qT_full = np.ascontiguousarray(q[0].T)    # [DM, L]
    kvT = np.ascontiguousarray(kv[0].T)       # [DM, L]
    Wq = np.ascontiguousarray(Wq, dtype=np.float32)
    Wkv = np.ascontiguousarray(Wkv, dtype=np.float32)
    Wc = np.ascontiguousarray(Wc, dtype=np.float32)

    in_maps = []
    for i in range(N_CORES):
        in_maps.append({
            "qT": np.ascontiguousarray(qT_full[:, LS * i:LS * (i + 1)]),
            "kvT": kvT,
            "Wq": Wq,
            "Wkv": Wkv,
            "Wc": Wc,
        })

    nc = _get_nc()
    res = run_bass_kernel_spmd(nc, in_maps, list(range(N_CORES)))
    out = np.concatenate([res.results[i]["out"] for i in range(N_CORES)], axis=0)
    return out.reshape(1, L, DM).astype(np.float32)

